# revision 3
# baseline (speedup 1.0000x reference)
"""Multi-head attention (B=8, T=2048, C=256, H=4) on 8 NeuronCores.

Data-parallel over batch: core b computes batch element b end-to-end.

v2 changes vs the 205us baseline (which was jointly PE/ACT-bound:
PE 152us busy, ACT 131us of softmax exp):

  * exp is SPLIT between ScalarE (true Exp activation) and VectorE
    (Schraudolph bit-trick: int16(round(s*A + B)) bitcast as bf16 is
    2^(s*0.125/ln2) to within +-3%; errors average out over the 2048-key
    softmax -- measured contribution ~5e-4 abs on HW probe).
  * PV matmuls drop the sumexp ones-column (M=64, not 65) and run the
    two heads of a pair CONCURRENTLY via PE column tiling
    (tile_position (0,0)/(0,64)); out2 for a head pair is one
    [128, 512] PSUM tile.
  * sumexp instead accumulates in a dedicated [97, 512] PSUM tile via
    4-way column-tiled K=128->M=1 ones-matmuls (one per head, col
    strips 0/32/64/96), riding PSUM accumulation across all 16 k-chunks.
  * x is pre-cast to bf16 on the host and transposed by the DMA xbar
    (dma_start_transpose) -- stage A costs zero PE/DVE time.
  * stage B bias rides ScalarE (Identity+per-partition bias).
  * normalization: 2 concurrent K=1 broadcast matmuls fill a [128,512]
    PSUM tile with 1/sumexp; one scalar_tensor_tensor fuses
    normalize+copy into yt (bf16).

Scores/PSUM accumulation stay fp32. Softmax skips max-subtraction:
logits ~N(0, 1/3) so exp() is in range.
"""

import numpy as np
import ml_dtypes

import concourse.bass as bass
import concourse.tile as tile
from concourse import bacc, mybir
from concourse.bass_utils import run_bass_kernel_spmd

B, T, C = 8, 2048, 256
H, HD = 4, 64
N_CORES = 8
F32 = mybir.dt.float32
F32R = mybir.dt.float32r
BF16 = mybir.dt.bfloat16
I16 = mybir.dt.int16

QT = 512                # q-tile (columns per score matmul)
NQT = T // QT           # 4
KC = T // 128           # 16 k-chunks of 128

# Schraudolph exp for bf16-bitcast: bf16(i16) ~= 2^(i16/128 - 127)
# want exp(s*0.125) = 2^(s*0.125/ln2):  i16 = s*(16/ln2) + 127*128 - c
EXP_A = float(np.float32(128.0 * 0.125 / np.log(2.0)))
EXP_B = float(np.float32(16256.0 - 7.5))

# which k-chunks use the DVE Schraudolph exp (rest use ScalarE true exp)
DVE_CHUNKS = frozenset((1, 3, 5, 7, 9, 11, 13))


def build_nc():
    nc = bacc.Bacc("TRN2", target_bir_lowering=False, debug=False,
                   num_devices=N_CORES)

    x_ap = nc.dram_tensor("xbf", [T, C], BF16, kind="ExternalInput").ap()
    wqk_ap = nc.dram_tensor("w_qkT", [C, 2 * C], F32R, kind="ExternalInput").ap()
    wv_ap = nc.dram_tensor("w_vT", [C, C], F32R, kind="ExternalInput").ap()
    wp_ap = nc.dram_tensor("w_pT", [C, C], F32R, kind="ExternalInput").ap()
    bqk_ap = nc.dram_tensor("b_qk", [4, 128], F32, kind="ExternalInput").ap()
    bv_ap = nc.dram_tensor("b_v", [C], F32, kind="ExternalInput").ap()
    bp_ap = nc.dram_tensor("b_p", [C], F32, kind="ExternalInput").ap()
    out_ap = nc.dram_tensor("out", [T, C], F32, kind="ExternalOutput").ap()

    with tile.TileContext(nc) as tc:
        with (
            tc.tile_pool(name="consts", bufs=1) as consts,
            tc.tile_pool(name="xt", bufs=1) as xtp,
            tc.tile_pool(name="qkt", bufs=1) as qktp,
            tc.tile_pool(name="vsb", bufs=1) as vsbp,
            tc.tile_pool(name="expa", bufs=3) as expa,
            tc.tile_pool(name="expd", bufs=3) as expd,
            tc.tile_pool(name="yt", bufs=1) as ytp,
            tc.tile_pool(name="small", bufs=2) as small,
            tc.tile_pool(name="ostage", bufs=4) as ostage,
            tc.tile_pool(name="scps", bufs=2, space="PSUM") as scps,
            tc.tile_pool(name="o2ps", bufs=1, space="PSUM") as o2ps,
            tc.tile_pool(name="seps", bufs=1, space="PSUM") as seps,
            tc.tile_pool(name="bcps", bufs=1, space="PSUM") as bcpsp,
        ):
            # ---- constants / weights -------------------------------------
            ones_pe = consts.tile([128, 1], BF16, tag="ones_pe")
            nc.vector.memset(ones_pe[:], 1.0)
            ones_bc = consts.tile([97, 64], BF16, tag="ones_bc")
            nc.vector.memset(ones_bc[:], 1.0)

            w_qk = [consts.tile([128, 2 * C], BF16, tag=f"wqk{c}", name=f"wqk{c}") for c in range(2)]
            for c in range(2):
                nc.gpsimd.dma_start(w_qk[c][:], wqk_ap[128 * c:128 * (c + 1), :])
            w_v = [consts.tile([128, C], BF16, tag=f"wv{c}", name=f"wv{c}") for c in range(2)]
            for c in range(2):
                nc.gpsimd.dma_start(w_v[c][:], wv_ap[128 * c:128 * (c + 1), :])
            w_p = [consts.tile([128, C], BF16, tag=f"wp{c}", name=f"wp{c}") for c in range(2)]
            for c in range(2):
                nc.gpsimd.dma_start(w_p[c][:], wp_ap[128 * c:128 * (c + 1), :])

            b_qk = consts.tile([128, 4], F32, tag="bqk")
            nc.gpsimd.dma_start(b_qk[:], bqk_ap.rearrange("c p -> p c"))
            b_p = consts.tile([128, C], F32, tag="bp")
            bp_bc = bass.AP(tensor=bp_ap.tensor, offset=bp_ap.offset,
                            ap=[[0, 128]] + list(bp_ap.ap))
            nc.gpsimd.dma_start(b_p[:], bp_bc)
            b_v = consts.tile([128, C], F32, tag="bv")
            bv_bc = bass.AP(tensor=bv_ap.tensor, offset=bv_ap.offset,
                            ap=[[0, 128]] + list(bv_ap.ap))
            nc.gpsimd.dma_start(b_v[:], bv_bc)

            # ---- stage A: xT via DMA xbar transpose (bf16 in DRAM) -------
            xt = [xtp.tile([128, T], BF16, tag=f"xt{c}", name=f"xt{c}") for c in range(2)]
            for c in range(2):
                nc.sync.dma_start_transpose(xt[c][:], x_ap[:, 128 * c:128 * (c + 1)])

            # ---- stage B: qkT [2C, T] = w_qk.T @ xT + b_qk ---------------
            # m-chunk 0: heads 0,1 q | 1: heads 2,3 q | 2: heads 0,1 k | 3: heads 2,3 k
            qkt = [qktp.tile([128, T], BF16, tag=f"qkt{m}", name=f"qkt{m}") for m in range(4)]
            def stage_b(n, ms=(0, 2, 1, 3)):
                for m in ms:
                    ps = scps.tile([128, 2 * QT], F32, tag="sc", name=f"bps{m}")
                    for c in range(2):
                        nc.tensor.matmul(
                            ps[:, 0:QT], w_qk[c][:, 128 * m:128 * (m + 1)],
                            xt[c][:, QT * n:QT * (n + 1)],
                            start=(c == 0), stop=(c == 1))
                    nc.scalar.add(
                        qkt[m][:, QT * n:QT * (n + 1)], ps[:, 0:QT], b_qk[:, m:m + 1])

            stage_b(0, ms=(0, 2))

            # ---- stage C: v [T, H, 64] natural + bias --------------------
            vsb = [vsbp.tile([128, H, HD], BF16, tag=f"v{tt}", name=f"v{tt}") for tt in range(KC)]
            for tt in range(KC):
                ps = scps.tile([128, 2 * QT], F32, tag="sc", name="cps")
                for c in range(2):
                    nc.tensor.matmul(
                        ps[:, 0:C], xt[c][:, 128 * tt:128 * (tt + 1)], w_v[c][:],
                        start=(c == 0), stop=(c == 1))
                nc.vector.tensor_add(
                    vsb[tt][:],
                    ps[:, 0:C].rearrange("p (h d) -> p h d", h=H),
                    b_v[:].rearrange("p (h d) -> p h d", h=H))

            stage_b(0, ms=(1, 3))
            for n in range(1, NQT):
                stage_b(n)

            # ---- stage D: attention, qt outer / chunk / head-pair --------
            yt = [ytp.tile([128, T], BF16, tag=f"yt{hp}", name=f"yt{hp}") for hp in range(2)]
            for qt in range(NQT):
                seacc = seps.tile([97, QT], F32, tag="se", name="seacc")
                o2 = [o2ps.tile([128, QT], F32, tag=f"o2{hp}", name=f"o2{hp}") for hp in range(2)]
                for i in range(KC):
                    exs = []
                    for hp in range(2):
                        qT = qkt[hp]
                        kT = qkt[hp + 2]
                        sc = scps.tile([128, 2 * QT], F32, tag="sc", name="sc")
                        for h in range(2):
                            nc.tensor.matmul(
                                sc[:, QT * h:QT * (h + 1)],
                                kT[64 * h:64 * (h + 1), 128 * i:128 * (i + 1)],
                                qT[64 * h:64 * (h + 1), QT * qt:QT * (qt + 1)],
                                start=True, stop=True)
                        if i in DVE_CHUNKS:
                            exd = expd.tile([128, 2 * QT], I16, tag="exd")
                            nc.vector.tensor_scalar(
                                exd[:], sc[:], EXP_A, EXP_B,
                                mybir.AluOpType.mult, mybir.AluOpType.add)
                            ex = exd[:].bitcast(BF16)
                        else:
                            exa = expa.tile([128, 2 * QT], BF16, tag="exa")
                            nc.scalar.activation(
                                exa[:], sc[:],
                                mybir.ActivationFunctionType.Exp,
                                bias=0.0, scale=0.125)
                            ex = exa[:]
                        exs.append(ex)
                        for h in range(2):
                            nc.tensor.matmul(
                                o2[hp][64 * h:64 * (h + 1), :],
                                vsb[i][:, 2 * hp + h, :],
                                ex[:, QT * h:QT * (h + 1)],
                                start=(i == 0), stop=(i == KC - 1),
                                tile_position=(0, 64 * h))
                    for j in range(4):
                        hp, h = divmod(j, 2)
                        nc.tensor.matmul(
                            seacc[32 * j:32 * j + 1, :],
                            ones_pe[:],
                            exs[hp][:, QT * h:QT * (h + 1)],
                            start=(i == 0), stop=(i == KC - 1),
                            tile_position=(0, 32 * j))
                # ---- normalize: rec = 1/sumexp, broadcast, fuse into yt --
                rec_f = small.tile([97, QT], F32, tag="rec_f")
                nc.vector.reciprocal_approx_fast(rec_f[:], seacc[:])
                rec = small.tile([97, QT], BF16, tag="rec")
                nc.vector.tensor_copy(rec[:], rec_f[:])
                for hp in range(2):
                    bc = bcpsp.tile([128, QT], F32, tag="bc", name="bc")
                    for h in range(2):
                        p = 32 * (2 * hp + h)
                        nc.tensor.matmul(
                            bc[64 * h:64 * (h + 1), :],
                            ones_bc[p:p + 1, :], rec[p:p + 1, :],
                            start=True, stop=True,
                            tile_position=(p, 64 * h))
                    bcs = small.tile([128, QT], BF16, tag="bcs")
                    nc.scalar.copy(bcs[:], bc[:])
                    nc.vector.scalar_tensor_tensor(
                        yt[hp][:, QT * qt:QT * (qt + 1)],
                        o2[hp][:], 1.0, bcs[:],
                        mybir.AluOpType.mult, mybir.AluOpType.mult)
                # ---- proj for this q-tile --------------------------------
                for tt in range(qt * QT // 128, (qt + 1) * QT // 128):
                    ps = bcpsp.tile([128, QT], F32, tag="bc", name="pps")
                    for c in range(2):
                        nc.tensor.matmul(
                            ps[:, 0:C], yt[c][:, 128 * tt:128 * (tt + 1)], w_p[c][:],
                            start=(c == 0), stop=(c == 1))
                    ost = ostage.tile([128, C], F32, tag="ost")
                    nc.vector.tensor_add(ost[:], ps[:, 0:C], b_p[:])
                    nc.sync.dma_start(out_ap[128 * tt:128 * (tt + 1), :], ost[:])
    nc.compile()
    return nc


_NC_CACHE = []


def _get_nc():
    if not _NC_CACHE:
        _NC_CACHE.append(build_nc())
    return _NC_CACHE[0]


def make_in_maps(x, w_qkv, b_qkv, w_proj, b_proj):
    shared = {
        "w_qkT": np.ascontiguousarray(w_qkv[:2 * C].T, dtype=np.float32),
        "w_vT": np.ascontiguousarray(w_qkv[2 * C:].T, dtype=np.float32),
        "w_pT": np.ascontiguousarray(w_proj.T, dtype=np.float32),
        "b_qk": np.ascontiguousarray(b_qkv[:2 * C].reshape(4, 128), dtype=np.float32),
        "b_v": np.ascontiguousarray(b_qkv[2 * C:], dtype=np.float32),
        "b_p": np.ascontiguousarray(b_proj, dtype=np.float32),
    }
    xbf = np.asarray(x, dtype=np.float32).astype(ml_dtypes.bfloat16)
    return [dict(shared, xbf=np.ascontiguousarray(xbf[b])) for b in range(B)]


def run(x, w_qkv, b_qkv, w_proj, b_proj, trace=False):
    nc = _get_nc()
    in_maps = make_in_maps(np.asarray(x), np.asarray(w_qkv), np.asarray(b_qkv),
                           np.asarray(w_proj), np.asarray(b_proj))
    res = run_bass_kernel_spmd(nc, in_maps, list(range(N_CORES)), trace=trace)
    out = np.stack([res.results[b]["out"] for b in range(B)])
    return out, res


def kernel(x, w_qkv, b_qkv, w_proj, b_proj):
    out, _ = run(x, w_qkv, b_qkv, w_proj, b_proj, trace=False)
    return out
